# revision 14
# baseline (speedup 1.0000x reference)
"""Trainium2 Bass kernel for nn_Encoder_57062935494680 (GAT-style GNN encoder).

Strategy (8 NeuronCores, node-sharded SPMD, single launch):
  - Nodes partitioned into 8 contiguous blocks of N/8. Weights replicated.
  - GAT edge softmax is reformulated densely: a host-built edge-count matrix
    C^T [src, dst] (bf16, exact small ints) turns gather/scatter segment ops
    into dense matmuls:  agg[d] = sum_s C[s,d]*exp(lrelu(a_s+a_d)) * h[s] / z[d].
    (Mathematically identical to the reference's segment softmax; the max
    subtraction is dropped since |logits| <= ~15 is safe in fp32.)
  - Per-encode AllGathers (h + a_src packed into extra payload rows) overlap
    with the other encode's compute; one more AllGather for h2.
  - h2^T for the decoder is rebuilt on-device by PE-transposing the gathered
    h2 blocks (cheaper than a second collective).
  - Batchnorm stats are computed redundantly on every core from gathered h2
    (streaming reduction, no rank-dependent indexing); each core then decodes
    only its own node block from its local h2^T.
  - The leaky-relu+bias of the attention logits runs as a custom fused DVE op
    on most s-tiles, with a fraction on ACT (Prelu) to balance engine load.

kernel(**inputs) takes FULL inputs, returns the FULL 6-tuple
(h2, h3, ret, ret_a, h2, h2_a) matching reference.reference().
"""
import os
import sys

sys.path.insert(0, "/opt/trn_rl_repo")

import numpy as np
import ml_dtypes

BF16 = ml_dtypes.bfloat16

# ---- custom fused DVE op: out = leaky_relu(in0 + s0) with slope imm2
from concourse import dve_ops as _dve_ops
from concourse.dve_spec import Spec as _Spec, Src0 as _Src0, C0 as _C0, C2 as _C2, maxx as _maxx

_v = _Src0 + _C0
_LRELU_SPEC = _Spec(body=_maxx(_v, _v * _C2))
LRELU_BIAS = _dve_ops.DveOp(
    "LRELU_BIAS", _LRELU_SPEC, subdim=False,
    uops_sha={"v3": "4e9e8c0667e5de9d", "v4": "ed815e8385e43fbf"})
if "LRELU_BIAS" not in _dve_ops._SUB_OPCODE_FOR_NAME:
    _dve_ops.OPS.append(LRELU_BIAS)
    _dve_ops.CUSTOM_DVE_SPECS["LRELU_BIAS"] = _LRELU_SPEC
    _dve_ops._SUB_OPCODE_FOR_NAME["LRELU_BIAS"] = (
        _dve_ops._CUSTOM_DVE_ROW_BASE + len(_dve_ops.OPS) - 1)

# ---------------------------------------------------------------- sizes
SMALL = bool(int(os.environ.get("BASS_GNN_SMALL", "0")))
NCORE = 8
if SMALL:
    N, IN, E = 1024, 384, 8192
else:
    N, IN, E = 8192, 3000, 262144
HID, OUT = 256, 64
EPS = 1e-5
NLOC = N // NCORE          # nodes per core
MS = NLOC // 128           # 128-row subtiles per core
ST = N // 128              # 128-row s-tiles over all nodes
WHE = 264                  # per-encode hcat width (256 h + 1 ones + pad)
HROWS = NLOC + 4           # + 4 rows carrying packed a_src
WH2 = 144                  # h2cat width (64+64+1 = 129, padded)
KT = [(k, min(128, IN - k)) for k in range(0, IN, 128)]   # IN k-tiles
NCH = [(c, min(500, IN - c)) for c in range(0, IN, 500)]  # h3 out chunks
ACT_EVERY = 5              # 1-in-5 s-tiles use the ACT(Prelu) path

_RUNNER = None


# ================================================================ device code
def _build_bass():
    import concourse.bass as bass
    import concourse.mybir as mybir
    import concourse.tile as tile
    from concourse import bacc
    from concourse.masks import make_identity

    dt = mybir.dt
    AF = mybir.ActivationFunctionType
    OP = mybir.AluOpType
    f32, bf16 = dt.float32, dt.bfloat16
    AX = mybir.AxisListType.X

    nc = bacc.Bacc(None, target_bir_lowering=False, debug=False)

    # ---------------- per-core external inputs
    featT0 = nc.declare_dram_parameter("featT0", [IN, NLOC], bf16, isOutput=False)
    featT1 = nc.declare_dram_parameter("featT1", [IN, NLOC], bf16, isOutput=False)
    ct_s = nc.declare_dram_parameter("ct_s", [N, NLOC], bf16, isOutput=False)
    gnT_s = nc.declare_dram_parameter("gnT_s", [N, NLOC], bf16, isOutput=False)
    W1bf = nc.declare_dram_parameter("W1bf", [IN, HID], bf16, isOutput=False)
    att = nc.declare_dram_parameter("att", [2 * HID], f32, isOutput=False)
    W2 = nc.declare_dram_parameter("W2", [HID, OUT], f32, isOutput=False)
    Wd1 = nc.declare_dram_parameter("Wd1", [OUT, HID], f32, isOutput=False)
    Wd1bf = nc.declare_dram_parameter("Wd1bf", [OUT, HID], bf16, isOutput=False)
    bd1 = nc.declare_dram_parameter("bd1", [HID], f32, isOutput=False)
    gamma = nc.declare_dram_parameter("gamma", [HID], f32, isOutput=False)
    beta = nc.declare_dram_parameter("beta", [HID], f32, isOutput=False)
    Wd2bf = nc.declare_dram_parameter("Wd2bf", [HID, IN], bf16, isOutput=False)
    bd2 = nc.declare_dram_parameter("bd2", [IN], f32, isOutput=False)
    discW = nc.declare_dram_parameter("discW", [OUT, OUT], f32, isOutput=False)
    discb = nc.declare_dram_parameter("discb", [1], f32, isOutput=False)

    # ---------------- per-core outputs
    out_h2 = nc.declare_dram_parameter("out_h2", [NLOC, OUT], f32, isOutput=True)
    out_h2a = nc.declare_dram_parameter("out_h2a", [NLOC, OUT], f32, isOutput=True)
    out_h3 = nc.declare_dram_parameter("out_h3", [NLOC, IN], f32, isOutput=True)
    out_ret = nc.declare_dram_parameter("out_ret", [NLOC, 2], f32, isOutput=True)
    out_reta = nc.declare_dram_parameter("out_reta", [NLOC, 2], f32, isOutput=True)

    # ---------------- internal DRAM (collective staging)
    hcat0 = nc.dram_tensor("hcat0", [HROWS, WHE], bf16)
    hcat1 = nc.dram_tensor("hcat1", [HROWS, WHE], bf16)
    hfull0 = nc.dram_tensor("hfull0", [NCORE * HROWS, WHE], bf16, addr_space="Shared")
    hfull1 = nc.dram_tensor("hfull1", [NCORE * HROWS, WHE], bf16, addr_space="Shared")
    adst0 = nc.dram_tensor("adst0", [NLOC], f32)
    adst1 = nc.dram_tensor("adst1", [NLOC], f32)
    h2cat = nc.dram_tensor("h2cat", [NLOC, WH2], bf16)
    h2full = nc.dram_tensor("h2full", [N, WH2], bf16, addr_space="Shared")

    RG = [list(range(NCORE))]
    hcats = (hcat0, hcat1)
    hfulls = (hfull0, hfull1)
    adsts = (adst0, adst1)

    with tile.TileContext(nc) as tc:
        with (
            tc.tile_pool(name="pconst", bufs=1) as pc,
            tc.tile_pool(name="ppersist", bufs=1) as pp,
            tc.tile_pool(name="pbig", bufs=4) as pb,
            tc.tile_pool(name="psml", bufs=3) as pm,
            tc.tile_pool(name="pP", bufs=3) as pPp,
            tc.tile_pool(name="pev", bufs=2) as pe,
            tc.tile_pool(name="ps", bufs=8, space="PSUM") as ps,
        ):
            def psum():
                return ps.tile([128, 512], f32, tag="ps", name="pst")

            # ======== setup: constants & broadcast helpers
            ident = pc.tile([128, 128], f32)
            make_identity(nc, ident)
            ident_bf = pc.tile([128, 128], bf16)
            make_identity(nc, ident_bf)
            ones1p = pc.tile([1, 128], f32)
            nc.vector.memset(ones1p[:], 1.0)
            onescol_bf = pc.tile([128, 1], bf16)
            nc.vector.memset(onescol_bf[:], 1.0)

            att_row = pc.tile([1, 2 * HID], f32)
            nc.sync.dma_start(att_row[:], att[None, :])
            att_bc = pc.tile([128, 2 * HID], f32)
            pbb = psum()
            nc.tensor.matmul(pbb[:, :], ones1p[:], att_row[:, 0:512], start=True, stop=True)
            nc.vector.tensor_copy(att_bc[:, 0:512], pbb[:, :])

            bd2_row = pc.tile([1, IN], f32)
            nc.sync.dma_start(bd2_row[:], bd2[None, :])
            bd2_bc = pc.tile([128, IN], bf16)
            for c0, cw in NCH:
                pbb = psum()
                nc.tensor.matmul(pbb[:, :cw], ones1p[:], bd2_row[:, c0:c0 + cw], start=True, stop=True)
                nc.vector.tensor_copy(bd2_bc[:, c0:c0 + cw], pbb[:, :cw])

            db_row = pc.tile([1, 1], f32)
            nc.sync.dma_start(db_row[:], discb[None, :])
            db_bc = pc.tile([128, 1], f32)
            pbb = psum()
            nc.tensor.matmul(pbb[:, :1], ones1p[:], db_row[:], start=True, stop=True)
            nc.vector.tensor_copy(db_bc[:], pbb[:, :1])

            # small weights
            Wd1_sb = pc.tile([OUT, HID], f32)
            nc.sync.dma_start(Wd1_sb[:], Wd1[:, :])
            Wd1b_sb = pc.tile([OUT, HID], bf16)
            nc.sync.dma_start(Wd1b_sb[:], Wd1bf[:, :])
            W2_sb = pc.tile([128, 2, OUT], f32)
            nc.sync.dma_start(W2_sb[:], W2.rearrange("(m p) o -> p m o", p=128))
            discW_sb = pc.tile([OUT, OUT], f32)
            nc.sync.dma_start(discW_sb[:], discW[:, :])
            bd1_sb = pc.tile([128, 2], f32)
            nc.sync.dma_start(bd1_sb[:], bd1.rearrange("(m p) -> p m", p=128))
            gamma_sb = pc.tile([128, 2], f32)
            nc.sync.dma_start(gamma_sb[:], gamma.rearrange("(m p) -> p m", p=128))
            beta_sb = pc.tile([128, 2], f32)
            nc.sync.dma_start(beta_sb[:], beta.rearrange("(m p) -> p m", p=128))

            # ======== phase A (per encode) + its AllGather
            for e in range(2):
                featT_e, adst_e, hcat_e = (featT0, featT1)[e], adsts[e], hcats[e]
                ph = [psum() for _ in range(MS)]
                for ki, (k0, kw) in enumerate(KT):
                    ft = pb.tile([128, NLOC], bf16, tag="ft")
                    nc.sync.dma_start(ft[:kw, :], featT_e[k0:k0 + kw, :])
                    wt = pm.tile([128, HID], bf16, tag="wt")
                    nc.sync.dma_start(wt[:kw, :], W1bf[k0:k0 + kw, :])
                    for m in range(MS):
                        nc.tensor.matmul(
                            ph[m][:, :HID],
                            ft[:kw, m * 128:(m + 1) * 128],
                            wt[:kw, :],
                            start=(ki == 0), stop=(ki == len(KT) - 1),
                        )
                for m in range(MS):
                    h_sb = pe.tile([128, HID], f32, tag="h_sb")
                    nc.scalar.activation(h_sb[:], ph[m][:, :HID], AF.Copy)
                    hbf = pe.tile([128, HID], bf16, tag="hbf")
                    nc.vector.tensor_copy(hbf[:], ph[m][:, :HID])
                    nc.sync.dma_start(hcat_e[m * 128:(m + 1) * 128, 0:HID], hbf[:])
                    nc.sync.dma_start(hcat_e[m * 128:(m + 1) * 128, HID:HID + 1], onescol_bf[:])
                    # attention logits for this node block; a_src rides in the
                    # hcat payload rows, a_dst stays local
                    tmp = pe.tile([128, HID], f32, tag="tmp")
                    nc.vector.tensor_mul(tmp[:], h_sb[:], att_bc[:, 0:HID])
                    asrf = pe.tile([128, 1], f32, tag="asrf")
                    nc.vector.reduce_sum(asrf[:], tmp[:], axis=AX)
                    asr = pe.tile([128, 1], bf16, tag="asr")
                    nc.vector.tensor_copy(asr[:], asrf[:])
                    nc.sync.dma_start(
                        hcat_e[NLOC + m // 2:NLOC + m // 2 + 1,
                               (m % 2) * 128:(m % 2) * 128 + 128].rearrange("one c -> c one"),
                        asr[:])
                    nc.vector.tensor_mul(tmp[:], h_sb[:], att_bc[:, HID:2 * HID])
                    ads = pe.tile([128, 1], f32, tag="ads")
                    nc.vector.reduce_sum(ads[:], tmp[:], axis=AX)
                    nc.sync.dma_start(adst_e[m * 128:(m + 1) * 128][:, None], ads[:])
                nc.gpsimd.collective_compute(
                    "AllGather", OP.bypass, ins=[hcat_e[:]], outs=[hfulls[e][:]],
                    replica_groups=RG)

            # ======== phase B: dense edge-softmax aggregation (per encode)
            h1T_sb = [pp.tile([128, 2, NLOC], f32, tag=f"h1T{e}", name=f"h1T{e}") for e in range(2)]
            NPAR = [(MS + 1) // 2, MS // 2]  # m-count per parity
            for e in range(2):
                hfull_e = hfulls[e]
                # a_src columns [p, st] from the packed rows of each rank block
                asrc_bf = pm.tile([128, ST], bf16, tag="asrc_bf")
                for r in range(NCORE):
                    base = r * HROWS
                    for q in range(min(2, MS)):
                        npar = NPAR[q]
                        if npar == 0:
                            continue
                        nc.gpsimd.dma_start(
                            asrc_bf[:, r * MS + q:r * MS + 2 * npar - (1 - q):2],
                            hfull_e[base + NLOC:base + NLOC + npar,
                                    q * 128:(q + 1) * 128].rearrange("u p -> p u"))
                asrc_sb = pc.tile([128, ST], f32, tag=f"asrc{e}", name=f"asrc{e}")
                nc.vector.tensor_copy(asrc_sb[:], asrc_bf[:])
                # a_dst broadcast rows (own nodes)
                row = pm.tile([1, NLOC], f32, tag="adrow")
                nc.sync.dma_start(row[:], adsts[e][None, :])
                adst_bc = pc.tile([128, NLOC], f32, tag=f"adbc{e}", name=f"adbc{e}")
                for c in range(0, NLOC, 512):
                    cw = min(512, NLOC - c)
                    pbb = psum()
                    nc.tensor.matmul(pbb[:, :cw], ones1p[:], row[:, c:c + cw], start=True, stop=True)
                    nc.vector.tensor_copy(adst_bc[:, c:c + cw], pbb[:, :cw])

                pg = [psum() for _ in range(MS)]
                for st in range(ST):
                    r, sub = st // MS, st % MS
                    base = r * HROWS
                    ct = pb.tile([128, NLOC], bf16, tag="ct")
                    nc.sync.dma_start(ct[:], ct_s[st * 128:(st + 1) * 128, :])
                    rhs = pm.tile([128, HID + 1], bf16, tag="rhs")
                    nc.gpsimd.dma_start(rhs[:], hfull_e[base + sub * 128:base + sub * 128 + 128, 0:HID + 1])
                    Xt = pPp.tile([128, NLOC], bf16, tag="Xt")
                    if st % ACT_EVERY == 0:
                        Et = pPp.tile([128, NLOC], f32, tag="Et")
                        nc.scalar.activation(Et[:], adst_bc[:], AF.Prelu,
                                             bias=asrc_sb[:, st:st + 1], alpha=0.2)
                        nc.scalar.activation(Xt[:], Et[:], AF.Exp)
                    else:
                        Wt = pPp.tile([128, NLOC], f32, tag="Et")
                        nc.vector._custom_dve(LRELU_BIAS, out=Wt[:], in0=adst_bc[:],
                                              s0=asrc_sb[:, st:st + 1], imm2=0.2)
                        nc.scalar.activation(Xt[:], Wt[:], AF.Exp)
                    Pt = pPp.tile([128, NLOC], bf16, tag="Pt")
                    nc.vector.tensor_mul(Pt[:], Xt[:], ct[:])
                    for m in range(MS):
                        nc.tensor.matmul(
                            pg[m][:, :HID + 1],
                            Pt[:, m * 128:(m + 1) * 128],
                            rhs[:],
                            start=(st == 0), stop=(st == ST - 1),
                        )
                # evict: alpha-normalize, elu, transpose -> h1T
                for m in range(MS):
                    zeps = pe.tile([128, 1], f32, tag="zeps")
                    nc.vector.tensor_scalar_add(zeps[:], pg[m][:, HID:HID + 1], 1e-16)
                    rz = pe.tile([128, 1], f32, tag="rz")
                    nc.vector.reciprocal(rz[:], zeps[:])
                    nm = pe.tile([128, HID], f32, tag="nm")
                    nc.vector.tensor_scalar(nm[:], pg[m][:, 0:HID], rz[:], None, OP.mult)
                    emn = pe.tile([128, HID], f32, tag="emn")
                    nc.vector.tensor_scalar_min(emn[:], nm[:], 0.0)
                    nc.scalar.activation(emn[:], emn[:], AF.Exp)
                    epo = pe.tile([128, HID], f32, tag="epo")
                    nc.vector.tensor_scalar(epo[:], nm[:], 0.0, -1.0, OP.max, OP.add)
                    h1 = pe.tile([128, HID], f32, tag="h1")
                    nc.vector.tensor_add(h1[:], emn[:], epo[:])
                    for kt2 in range(2):
                        pt = psum()
                        nc.tensor.transpose(pt[:, :128], h1[:, kt2 * 128:(kt2 + 1) * 128], ident[:])
                        nc.vector.tensor_copy(h1T_sb[e][:, kt2, m * 128:(m + 1) * 128], pt[:, :128])

            # ======== phase C: h2 = h1 @ W2 (+ local transposes, AG#2 staging)
            h2T_sb = [pp.tile([OUT, NLOC], f32, tag=f"h2T{e}", name=f"h2T{e}") for e in range(2)]
            for e in range(2):
                for m in range(MS):
                    p2 = psum()
                    for kt2 in range(2):
                        nc.tensor.matmul(
                            p2[:, :OUT],
                            h1T_sb[e][:, kt2, m * 128:(m + 1) * 128],
                            W2_sb[:, kt2, :],
                            start=(kt2 == 0), stop=(kt2 == 1),
                        )
                    h2s = pe.tile([128, OUT], f32, tag="h2s")
                    nc.scalar.activation(h2s[:], p2[:, :OUT], AF.Copy)
                    nc.sync.dma_start(
                        (out_h2 if e == 0 else out_h2a)[m * 128:(m + 1) * 128, :], h2s[:])
                    h2b = pe.tile([128, OUT], bf16, tag="h2b")
                    nc.vector.tensor_copy(h2b[:], p2[:, :OUT])
                    nc.sync.dma_start(h2cat[m * 128:(m + 1) * 128, e * OUT:(e + 1) * OUT], h2b[:])
                    if e == 0:
                        nc.sync.dma_start(h2cat[m * 128:(m + 1) * 128, 128:129], onescol_bf[:])
                    ptr = psum()
                    nc.tensor.transpose(ptr[:OUT, :128], h2s[:, :], ident[:])
                    nc.vector.tensor_copy(h2T_sb[e][:, m * 128:(m + 1) * 128], ptr[:OUT, :128])

            # ======== AllGather #2
            nc.gpsimd.collective_compute(
                "AllGather", OP.bypass, ins=[h2cat[:]], outs=[h2full[:]], replica_groups=RG)

            # ======== phase E: readout + discriminator
            prb = [psum() for _ in range((MS + 1) // 2)]
            pr = [prb[m // 2][:, (m % 2) * 256:(m % 2) * 256 + WH2] for m in range(MS)]
            for st in range(ST):
                gt = pb.tile([128, NLOC], bf16, tag="gt")
                nc.sync.dma_start(gt[:], gnT_s[st * 128:(st + 1) * 128, :])
                rr = pm.tile([128, WH2], bf16, tag="rr")
                nc.gpsimd.dma_start(rr[:], h2full[st * 128:(st + 1) * 128, :])
                for m in range(MS):
                    nc.tensor.matmul(
                        pr[m],
                        gt[:, m * 128:(m + 1) * 128],
                        rr[:],
                        start=(st == 0), stop=(st == ST - 1))
            # sweep 1: normalize g (Sqrt ops batched on ACT)
            gall = pp.tile([128, 2 * MS, OUT], f32)
            for m in range(MS):
                rm = pe.tile([128, 1], f32, tag="rm")
                nc.vector.reciprocal(rm[:], pr[m][:, 128:129])
                for gi in range(2):
                    g = gall[:, gi * MS + m, :]
                    nc.vector.tensor_scalar(g, pr[m][:, gi * OUT:(gi + 1) * OUT], rm[:], None, OP.mult)
                    sq = pe.tile([128, OUT], f32, tag="sq")
                    nc.vector.tensor_mul(sq[:], g, g)
                    ssn = pe.tile([128, 1], f32, tag="ssn")
                    nc.vector.reduce_sum(ssn[:], sq[:], axis=AX)
                    nc.scalar.activation(ssn[:], ssn[:], AF.Sqrt)
                    nc.vector.tensor_scalar_max(ssn[:], ssn[:], 1e-12)
                    nc.vector.reciprocal(ssn[:], ssn[:])
                    nc.vector.tensor_scalar(g, g, ssn[:], None, OP.mult)
            # sweep 2: one batched sigmoid
            nc.scalar.activation(gall[:], gall[:], AF.Sigmoid)
            # sweep 3: discriminator
            for m in range(MS):
                ts_ = []
                for e in range(2):
                    pd = psum()
                    nc.tensor.matmul(
                        pd[:, :OUT],
                        h2T_sb[e][:, m * 128:(m + 1) * 128],
                        discW_sb[:],
                        start=True, stop=True)
                    tt = pe.tile([128, OUT], f32, tag=f"tt{e}", name=f"tt{e}")
                    nc.vector.tensor_copy(tt[:], pd[:, :OUT])
                    ts_.append(tt)
                g0 = gall[:, 0 * MS + m, :]
                ga = gall[:, 1 * MS + m, :]
                prod = pe.tile([128, OUT], f32, tag="prod")
                retv = pe.tile([128, 2], f32, tag="retv")
                retav = pe.tile([128, 2], f32, tag="retav")
                for tv, gv, dtile, dcol in (
                        (ts_[0], g0, retv, 0), (ts_[1], g0, retv, 1),
                        (ts_[1], ga, retav, 0), (ts_[0], ga, retav, 1)):
                    nc.vector.tensor_mul(prod[:], tv[:], gv)
                    nc.vector.reduce_sum(dtile[:, dcol:dcol + 1], prod[:], axis=AX)
                nc.vector.tensor_scalar_add(retv[:], retv[:], db_bc[:])
                nc.vector.tensor_scalar_add(retav[:], retav[:], db_bc[:])
                nc.sync.dma_start(out_ret[m * 128:(m + 1) * 128, :], retv[:])
                nc.sync.dma_start(out_reta[m * 128:(m + 1) * 128, :], retav[:])

            # ======== phase D: decoder
            # rebuild h2^T (all nodes, bf16) by transposing gathered h2 blocks
            h2T_all = pp.tile([OUT, N], bf16)
            for stt in range(ST):
                h2f = pe.tile([128, OUT], bf16, tag="h2f")
                nc.gpsimd.dma_start(h2f[:], h2full[stt * 128:(stt + 1) * 128, 0:OUT])
                ptx = ps.tile([128, 512], bf16, tag="ps", name="pstb")
                nc.tensor.transpose(ptx[:OUT, :128], h2f[:], ident_bf[:])
                nc.vector.tensor_copy(h2T_all[:, stt * 128:(stt + 1) * 128], ptx[:OUT, :128])
            # streaming BN stats over all nodes
            NCHZ = [(c, min(512, N - c)) for c in range(0, N, 512)]
            nzch = len(NCHZ)
            acc_s = pc.tile([128, 2, nzch], f32)
            acc_q = pc.tile([128, 2, nzch], f32)
            for ci, (c0, cw) in enumerate(NCHZ):
                for m2 in range(2):
                    pz = psum()
                    nc.tensor.matmul(
                        pz[:, :cw],
                        Wd1b_sb[:, m2 * 128:(m2 + 1) * 128],
                        h2T_all[:, c0:c0 + cw],
                        start=True, stop=True)
                    nc.vector.reduce_sum(acc_s[:, m2, ci:ci + 1], pz[:, :cw], axis=AX)
                    sqs = pe.tile([128, 512], f32, tag="h3s")
                    nc.scalar.activation(sqs[:, :cw], pz[:, :cw], AF.Square,
                                         accum_out=acc_q[:, m2, ci:ci + 1])
            # stats -> scale/shift
            ssum = pc.tile([128, 2], f32)
            qsum = pc.tile([128, 2], f32)
            for m2 in range(2):
                nc.vector.reduce_sum(ssum[:, m2:m2 + 1], acc_s[:, m2, :], axis=AX)
                nc.vector.reduce_sum(qsum[:, m2:m2 + 1], acc_q[:, m2, :], axis=AX)
            mur = pc.tile([128, 2], f32)
            nc.vector.tensor_scalar_mul(mur[:], ssum[:], 1.0 / N)
            var = pc.tile([128, 2], f32)
            msq = pe.tile([128, 2], f32, tag="msq")
            nc.vector.tensor_mul(msq[:], mur[:], mur[:])
            nc.vector.tensor_scalar_mul(var[:], qsum[:], 1.0 / N)
            nc.vector.tensor_sub(var[:], var[:], msq[:])
            eps_col = pc.tile([128, 1], f32)
            nc.vector.memset(eps_col[:], EPS)
            std = pc.tile([128, 2], f32)
            nc.scalar.activation(std[:], var[:], AF.Sqrt, bias=eps_col[:])
            rs = pc.tile([128, 2], f32)
            nc.vector.reciprocal(rs[:], std[:])
            scl = pc.tile([128, 2], f32)
            nc.vector.tensor_mul(scl[:], rs[:], gamma_sb[:])
            sh = pc.tile([128, 2], f32)
            nc.vector.tensor_sub(sh[:], bd1_sb[:], mur[:])
            nc.vector.tensor_mul(sh[:], sh[:], scl[:])
            nc.vector.tensor_add(sh[:], sh[:], beta_sb[:])
            # own-block z, normalize, elu
            NCHL = [(c, min(512, NLOC - c)) for c in range(0, NLOC, 512)]
            znT = pp.tile([128, 2, NLOC], f32)
            for m2 in range(2):
                for c0, cw in NCHL:
                    pz = psum()
                    nc.tensor.matmul(
                        pz[:, :cw],
                        Wd1_sb[:, m2 * 128:(m2 + 1) * 128],
                        h2T_sb[0][:, c0:c0 + cw],
                        start=True, stop=True)
                    nc.vector.tensor_scalar(
                        znT[:, m2, c0:c0 + cw], pz[:, :cw],
                        scl[:, m2:m2 + 1], sh[:, m2:m2 + 1], OP.mult, OP.add)
            zel = pp.tile([128, 2, NLOC], bf16)
            tmp1 = pp.tile([128, 2, NLOC], f32)
            nc.vector.tensor_scalar_min(tmp1[:], znT[:], 0.0)
            nc.scalar.activation(tmp1[:], tmp1[:], AF.Exp)
            nc.vector.tensor_scalar(zel[:], znT[:], 0.0, -1.0, OP.max, OP.add)
            nc.vector.tensor_add(zel[:], zel[:], tmp1[:])
            # h3 = zel^T @ Wd2 + bd2
            Wd2r = Wd2bf.rearrange("(m p) i -> p m i", p=128)
            for c0, cw in NCH:
                wd2t = pm.tile([128, 2, 500], bf16, tag="wd2t")
                nc.sync.dma_start(wd2t[:, :, :cw], Wd2r[:, :, c0:c0 + cw])
                p3 = [psum() for _ in range(MS)]
                for m in range(MS):
                    for kt2 in range(2):
                        nc.tensor.matmul(
                            p3[m][:, :cw],
                            zel[:, kt2, m * 128:(m + 1) * 128],
                            wd2t[:, kt2, :cw],
                            start=(kt2 == 0), stop=(kt2 == 1))
                    h3s = pe.tile([128, 512], f32, tag="h3s")
                    nc.vector.tensor_add(h3s[:, :cw], p3[m][:, :cw], bd2_bc[:, c0:c0 + cw])
                    nc.sync.dma_start(out_h3[m * 128:(m + 1) * 128, c0:c0 + cw], h3s[:, :cw])

    nc.compile()
    return nc


# Need mybir at module level for the runner
import concourse.mybir as mybir  # noqa: E402


# ================================================================ host runner
class _SpmdRunner:
    def __init__(self, nc, n_cores):
        import jax
        from jax.sharding import Mesh, PartitionSpec, NamedSharding
        from jax.experimental.shard_map import shard_map
        from concourse.bass2jax import (
            _bass_exec_p, install_neuronx_cc_hook, partition_id_tensor)

        install_neuronx_cc_hook()
        self.jax = jax
        self.nc = nc
        self.n_cores = n_cores
        partition_name = nc.partition_id_tensor.name if nc.partition_id_tensor else None
        in_names, out_names, out_avals, zero_shapes = [], [], [], []
        for alloc in nc.m.functions[0].allocations:
            if not isinstance(alloc, mybir.MemoryLocationSet):
                continue
            name = alloc.memorylocations[0].name
            if alloc.kind == "ExternalInput":
                if name != partition_name:
                    in_names.append(name)
            elif alloc.kind == "ExternalOutput":
                shape = tuple(alloc.tensor_shape)
                dtype = mybir.dt.np(alloc.dtype)
                out_names.append(name)
                out_avals.append(jax.core.ShapedArray(shape, dtype))
                zero_shapes.append((shape, dtype))
        self.in_names, self.out_names = in_names, out_names
        self.out_avals, self.zero_shapes = out_avals, zero_shapes
        n_params, n_outs = len(in_names), len(out_avals)
        all_in = in_names + out_names + ([partition_name] if partition_name else [])

        def _body(*args):
            operands = list(args)
            if partition_name is not None:
                operands.append(partition_id_tensor())
            outs = _bass_exec_p.bind(
                *operands, out_avals=tuple(out_avals), in_names=tuple(all_in),
                out_names=tuple(out_names), lowering_input_output_aliases=(),
                sim_require_finite=True, sim_require_nnan=True, nc=nc)
            return tuple(outs)

        devices = jax.devices()[:n_cores]
        self.mesh = Mesh(np.asarray(devices), ("core",))
        in_specs = (PartitionSpec("core"),) * (n_params + n_outs)
        out_specs = (PartitionSpec("core"),) * n_outs
        self.fn = jax.jit(
            shard_map(_body, mesh=self.mesh, in_specs=in_specs,
                      out_specs=out_specs, check_rep=False),
            keep_unused=True)
        self.sharding = NamedSharding(self.mesh, PartitionSpec("core"))

    def put_inputs(self, in_maps):
        jax = self.jax
        args = []
        for name in self.in_names:
            if name == "dbg_addr" and name not in in_maps[0]:
                per = [np.zeros((1, 2), np.uint32)] * self.n_cores
            else:
                per = [np.asarray(in_maps[c][name]) for c in range(self.n_cores)]
            args.append(jax.device_put(np.concatenate(per, axis=0), self.sharding))
        for shape, dtype in self.zero_shapes:
            z = np.zeros((self.n_cores * shape[0], *shape[1:]), dtype)
            args.append(jax.device_put(z, self.sharding))
        return args

    def run(self, args):
        outs = self.fn(*args)
        self.jax.block_until_ready(outs)
        return outs

    def results(self, outs):
        res = []
        for c in range(self.n_cores):
            d = {}
            for i, name in enumerate(self.out_names):
                shape = self.out_avals[i].shape
                d[name] = np.asarray(outs[i]).reshape(self.n_cores, *shape)[c]
            res.append(d)
        return res


def _get_runner():
    global _RUNNER
    if _RUNNER is None:
        nc = _build_bass()
        _RUNNER = _SpmdRunner(nc, NCORE)
    return _RUNNER


# ================================================================ host-side prep
def _prep_in_maps(feat, feat_a, graph_neigh, W1, att_src, att_dst, W2, Wd1, bd1,
                  gamma, beta, Wd2, bd2, disc_W, disc_b, edge_index):
    feat = np.asarray(feat, np.float32)
    feat_a = np.asarray(feat_a, np.float32)
    gn = np.asarray(graph_neigh, np.float32)
    ei = np.asarray(edge_index).astype(np.int64)
    src, dst = ei[0], ei[1]

    # dense transposed edge-count matrix C^T[s, d]
    CT = np.zeros((N, N), np.float32)
    np.add.at(CT, (src, dst), 1.0)
    assert CT.max() < 256, "bf16 exact-int range exceeded"

    featT = np.ascontiguousarray(feat.T).astype(BF16)
    feataT = np.ascontiguousarray(feat_a.T).astype(BF16)
    gnT_bf = np.ascontiguousarray(gn.T).astype(BF16)

    att = np.concatenate([np.asarray(att_src, np.float32),
                          np.asarray(att_dst, np.float32)])
    common = {
        "W1bf": np.asarray(W1, np.float32).astype(BF16), "att": att,
        "W2": np.asarray(W2, np.float32), "Wd1": np.asarray(Wd1, np.float32),
        "Wd1bf": np.asarray(Wd1, np.float32).astype(BF16),
        "bd1": np.asarray(bd1, np.float32), "gamma": np.asarray(gamma, np.float32),
        "beta": np.asarray(beta, np.float32),
        "Wd2bf": np.asarray(Wd2, np.float32).astype(BF16),
        "bd2": np.asarray(bd2, np.float32), "discW": np.asarray(disc_W, np.float32),
        "discb": np.asarray(disc_b, np.float32).reshape(1),
    }
    in_maps = []
    for r in range(NCORE):
        sl = slice(r * NLOC, (r + 1) * NLOC)
        m = dict(common)
        m["featT0"] = np.ascontiguousarray(featT[:, sl])
        m["featT1"] = np.ascontiguousarray(feataT[:, sl])
        m["ct_s"] = np.ascontiguousarray(CT[:, sl]).astype(BF16)
        m["gnT_s"] = np.ascontiguousarray(gnT_bf[:, sl])
        in_maps.append(m)
    return in_maps


def kernel(**inputs):
    runner = _get_runner()
    in_maps = _prep_in_maps(**inputs)
    args = runner.put_inputs(in_maps)
    outs = runner.run(args)
    res = runner.results(outs)
    H2 = np.concatenate([res[r]["out_h2"] for r in range(NCORE)], axis=0)
    H2A = np.concatenate([res[r]["out_h2a"] for r in range(NCORE)], axis=0)
    H3 = np.concatenate([res[r]["out_h3"] for r in range(NCORE)], axis=0)
    RET = np.concatenate([res[r]["out_ret"] for r in range(NCORE)], axis=0)
    RETA = np.concatenate([res[r]["out_reta"] for r in range(NCORE)], axis=0)
    return (H2, H3, RET, RETA, H2, H2A)


# revision 15
# speedup vs baseline: 1.6081x; 1.6081x over previous
"""Trainium2 Bass kernel for nn_Encoder_57062935494680 (GAT-style GNN encoder).

Strategy (8 NeuronCores, node-sharded SPMD, single launch):
  - Nodes partitioned into 8 contiguous blocks of N/8. Weights replicated.
  - GAT edge softmax is reformulated densely: a host-built edge-count matrix
    C^T [src, dst] (bf16, exact small ints) turns gather/scatter segment ops
    into dense matmuls:  agg[d] = sum_s C[s,d]*exp(lrelu(a_s+a_d)) * h[s] / z[d].
    (Mathematically identical to the reference's segment softmax; the max
    subtraction is dropped since |logits| <= ~15 is safe in fp32.)
  - Per-encode AllGathers (h + a_src packed into extra payload rows) overlap
    with the other encode's compute; one more AllGather for h2.
  - h2^T for the decoder is rebuilt on-device by PE-transposing the gathered
    h2 blocks (cheaper than a second collective).
  - Batchnorm stats are computed redundantly on every core from gathered h2
    (streaming reduction, no rank-dependent indexing); each core then decodes
    only its own node block from its local h2^T.
  - The leaky-relu+bias of the attention logits runs as a custom fused DVE op
    on most s-tiles, with a fraction on ACT (Prelu) to balance engine load.

kernel(**inputs) takes FULL inputs, returns the FULL 6-tuple
(h2, h3, ret, ret_a, h2, h2_a) matching reference.reference().
"""
import os
import sys

sys.path.insert(0, "/opt/trn_rl_repo")

import numpy as np
import ml_dtypes

BF16 = ml_dtypes.bfloat16

# ---- custom fused DVE op: out = leaky_relu(in0 + s0) with slope imm2
from concourse import dve_ops as _dve_ops
from concourse.dve_spec import Spec as _Spec, Src0 as _Src0, C0 as _C0, C2 as _C2, maxx as _maxx

_v = _Src0 + _C0
_LRELU_SPEC = _Spec(body=_maxx(_v, _v * _C2))
LRELU_BIAS = _dve_ops.DveOp(
    "LRELU_BIAS", _LRELU_SPEC, subdim=False,
    uops_sha={"v3": "4e9e8c0667e5de9d", "v4": "ed815e8385e43fbf"})
if "LRELU_BIAS" not in _dve_ops._SUB_OPCODE_FOR_NAME:
    _dve_ops.OPS.append(LRELU_BIAS)
    _dve_ops.CUSTOM_DVE_SPECS["LRELU_BIAS"] = _LRELU_SPEC
    _dve_ops._SUB_OPCODE_FOR_NAME["LRELU_BIAS"] = (
        _dve_ops._CUSTOM_DVE_ROW_BASE + len(_dve_ops.OPS) - 1)

# ---------------------------------------------------------------- sizes
SMALL = bool(int(os.environ.get("BASS_GNN_SMALL", "0")))
NCORE = 8
if SMALL:
    N, IN, E = 1024, 384, 8192
else:
    N, IN, E = 8192, 3000, 262144
HID, OUT = 256, 64
EPS = 1e-5
NLOC = N // NCORE          # nodes per core
MS = NLOC // 128           # 128-row subtiles per core
ST = N // 128              # 128-row s-tiles over all nodes
WHE = 264                  # per-encode hcat width (256 h + 1 ones + pad)
HROWS = NLOC + 4           # + 4 rows carrying packed a_src
WH2 = 144                  # h2cat width (64+64+1 = 129, padded)
KT = [(k, min(128, IN - k)) for k in range(0, IN, 128)]   # IN k-tiles
NCH = [(c, min(500, IN - c)) for c in range(0, IN, 500)]  # h3 out chunks
ABLATE = os.environ.get("BASS_GNN_ABLATE", "")
ACT_EVERY = 1 if ABLATE == "noact" else 5  # 1-in-N s-tiles use the ACT(Prelu) path

_RUNNER = None


# ================================================================ device code
def _build_bass():
    import concourse.bass as bass
    import concourse.mybir as mybir
    import concourse.tile as tile
    from concourse import bacc
    from concourse.masks import make_identity

    dt = mybir.dt
    AF = mybir.ActivationFunctionType
    OP = mybir.AluOpType
    f32, bf16 = dt.float32, dt.bfloat16
    AX = mybir.AxisListType.X

    nc = bacc.Bacc(None, target_bir_lowering=False, debug=False)

    # ---------------- per-core external inputs
    featT0 = nc.declare_dram_parameter("featT0", [IN, NLOC], bf16, isOutput=False)
    featT1 = nc.declare_dram_parameter("featT1", [IN, NLOC], bf16, isOutput=False)
    ct_s = nc.declare_dram_parameter("ct_s", [N, NLOC], bf16, isOutput=False)
    gnT_s = nc.declare_dram_parameter("gnT_s", [N, NLOC], bf16, isOutput=False)
    W1bf = nc.declare_dram_parameter("W1bf", [IN, HID], bf16, isOutput=False)
    att = nc.declare_dram_parameter("att", [2 * HID], f32, isOutput=False)
    W2 = nc.declare_dram_parameter("W2", [HID, OUT], f32, isOutput=False)
    Wd1 = nc.declare_dram_parameter("Wd1", [OUT, HID], f32, isOutput=False)
    Wd1bf = nc.declare_dram_parameter("Wd1bf", [OUT, HID], bf16, isOutput=False)
    bd1 = nc.declare_dram_parameter("bd1", [HID], f32, isOutput=False)
    gamma = nc.declare_dram_parameter("gamma", [HID], f32, isOutput=False)
    beta = nc.declare_dram_parameter("beta", [HID], f32, isOutput=False)
    Wd2bf = nc.declare_dram_parameter("Wd2bf", [HID, IN], bf16, isOutput=False)
    bd2 = nc.declare_dram_parameter("bd2", [IN], f32, isOutput=False)
    discW = nc.declare_dram_parameter("discW", [OUT, OUT], f32, isOutput=False)
    discb = nc.declare_dram_parameter("discb", [1], f32, isOutput=False)

    # ---------------- per-core outputs
    out_h2 = nc.declare_dram_parameter("out_h2", [NLOC, OUT], f32, isOutput=True)
    out_h2a = nc.declare_dram_parameter("out_h2a", [NLOC, OUT], f32, isOutput=True)
    out_h3 = nc.declare_dram_parameter("out_h3", [NLOC, IN], f32, isOutput=True)
    out_ret = nc.declare_dram_parameter("out_ret", [NLOC, 2], f32, isOutput=True)
    out_reta = nc.declare_dram_parameter("out_reta", [NLOC, 2], f32, isOutput=True)

    # ---------------- internal DRAM (collective staging)
    hcat0 = nc.dram_tensor("hcat0", [HROWS, WHE], bf16)
    hcat1 = nc.dram_tensor("hcat1", [HROWS, WHE], bf16)
    hfull0 = nc.dram_tensor("hfull0", [NCORE * HROWS, WHE], bf16, addr_space="Shared")
    hfull1 = nc.dram_tensor("hfull1", [NCORE * HROWS, WHE], bf16, addr_space="Shared")
    adst0 = nc.dram_tensor("adst0", [NLOC], f32)
    adst1 = nc.dram_tensor("adst1", [NLOC], f32)
    h2cat = nc.dram_tensor("h2cat", [NLOC, WH2], bf16)
    h2full = nc.dram_tensor("h2full", [N, WH2], bf16, addr_space="Shared")

    RG = [list(range(NCORE))]
    hcats = (hcat0, hcat1)
    hfulls = (hfull0, hfull1)
    adsts = (adst0, adst1)

    with tile.TileContext(nc) as tc:
        with (
            tc.tile_pool(name="pconst", bufs=1) as pc,
            tc.tile_pool(name="ppersist", bufs=1) as pp,
            tc.tile_pool(name="pbig", bufs=4) as pb,
            tc.tile_pool(name="psml", bufs=3) as pm,
            tc.tile_pool(name="pP", bufs=3) as pPp,
            tc.tile_pool(name="pev", bufs=2) as pe,
            tc.tile_pool(name="ps", bufs=8, space="PSUM") as ps,
        ):
            def psum():
                return ps.tile([128, 512], f32, tag="ps", name="pst")

            # ======== setup: constants & broadcast helpers
            ident = pc.tile([128, 128], f32)
            make_identity(nc, ident)
            ident_bf = pc.tile([128, 128], bf16)
            make_identity(nc, ident_bf)
            ones1p = pc.tile([1, 128], f32)
            nc.vector.memset(ones1p[:], 1.0)
            onescol_bf = pc.tile([128, 1], bf16)
            nc.vector.memset(onescol_bf[:], 1.0)

            att_row = pc.tile([1, 2 * HID], f32)
            nc.sync.dma_start(att_row[:], att[None, :])
            att_bc = pc.tile([128, 2 * HID], f32)
            pbb = psum()
            nc.tensor.matmul(pbb[:, :], ones1p[:], att_row[:, 0:512], start=True, stop=True)
            nc.vector.tensor_copy(att_bc[:, 0:512], pbb[:, :])

            bd2_row = pc.tile([1, IN], f32)
            nc.sync.dma_start(bd2_row[:], bd2[None, :])
            bd2_bc = pc.tile([128, IN], bf16)
            for c0, cw in NCH:
                pbb = psum()
                nc.tensor.matmul(pbb[:, :cw], ones1p[:], bd2_row[:, c0:c0 + cw], start=True, stop=True)
                nc.vector.tensor_copy(bd2_bc[:, c0:c0 + cw], pbb[:, :cw])

            db_row = pc.tile([1, 1], f32)
            nc.sync.dma_start(db_row[:], discb[None, :])
            db_bc = pc.tile([128, 1], f32)
            pbb = psum()
            nc.tensor.matmul(pbb[:, :1], ones1p[:], db_row[:], start=True, stop=True)
            nc.vector.tensor_copy(db_bc[:], pbb[:, :1])

            # small weights
            Wd1_sb = pc.tile([OUT, HID], f32)
            nc.sync.dma_start(Wd1_sb[:], Wd1[:, :])
            Wd1b_sb = pc.tile([OUT, HID], bf16)
            nc.sync.dma_start(Wd1b_sb[:], Wd1bf[:, :])
            W2_sb = pc.tile([128, 2, OUT], f32)
            nc.sync.dma_start(W2_sb[:], W2.rearrange("(m p) o -> p m o", p=128))
            discW_sb = pc.tile([OUT, OUT], f32)
            nc.sync.dma_start(discW_sb[:], discW[:, :])
            bd1_sb = pc.tile([128, 2], f32)
            nc.sync.dma_start(bd1_sb[:], bd1.rearrange("(m p) -> p m", p=128))
            gamma_sb = pc.tile([128, 2], f32)
            nc.sync.dma_start(gamma_sb[:], gamma.rearrange("(m p) -> p m", p=128))
            beta_sb = pc.tile([128, 2], f32)
            nc.sync.dma_start(beta_sb[:], beta.rearrange("(m p) -> p m", p=128))

            # ======== phase A (per encode) + its AllGather
            for e in range(2):
                featT_e, adst_e, hcat_e = (featT0, featT1)[e], adsts[e], hcats[e]
                ph = [psum() for _ in range(MS)]
                for ki, (k0, kw) in enumerate(KT):
                    ft = pb.tile([128, NLOC], bf16, tag="ft")
                    nc.sync.dma_start(ft[:kw, :], featT_e[k0:k0 + kw, :])
                    wt = pm.tile([128, HID], bf16, tag="wt")
                    nc.sync.dma_start(wt[:kw, :], W1bf[k0:k0 + kw, :])
                    for m in range(MS):
                        nc.tensor.matmul(
                            ph[m][:, :HID],
                            ft[:kw, m * 128:(m + 1) * 128],
                            wt[:kw, :],
                            start=(ki == 0), stop=(ki == len(KT) - 1),
                        )
                for m in range(MS):
                    h_sb = pe.tile([128, HID], f32, tag="h_sb")
                    nc.scalar.activation(h_sb[:], ph[m][:, :HID], AF.Copy)
                    hbf = pe.tile([128, HID], bf16, tag="hbf")
                    nc.vector.tensor_copy(hbf[:], ph[m][:, :HID])
                    nc.sync.dma_start(hcat_e[m * 128:(m + 1) * 128, 0:HID], hbf[:])
                    nc.sync.dma_start(hcat_e[m * 128:(m + 1) * 128, HID:HID + 1], onescol_bf[:])
                    # attention logits for this node block; a_src rides in the
                    # hcat payload rows, a_dst stays local
                    tmp = pe.tile([128, HID], f32, tag="tmp")
                    nc.vector.tensor_mul(tmp[:], h_sb[:], att_bc[:, 0:HID])
                    asrf = pe.tile([128, 1], f32, tag="asrf")
                    nc.vector.reduce_sum(asrf[:], tmp[:], axis=AX)
                    asr = pe.tile([128, 1], bf16, tag="asr")
                    nc.vector.tensor_copy(asr[:], asrf[:])
                    nc.sync.dma_start(
                        hcat_e[NLOC + m // 2:NLOC + m // 2 + 1,
                               (m % 2) * 128:(m % 2) * 128 + 128].rearrange("one c -> c one"),
                        asr[:])
                    nc.vector.tensor_mul(tmp[:], h_sb[:], att_bc[:, HID:2 * HID])
                    ads = pe.tile([128, 1], f32, tag="ads")
                    nc.vector.reduce_sum(ads[:], tmp[:], axis=AX)
                    nc.sync.dma_start(adst_e[m * 128:(m + 1) * 128][:, None], ads[:])
                if ABLATE == "nocc":
                    for _r in range(NCORE):
                        nc.gpsimd.dma_start(hfulls[e][_r * HROWS:(_r + 1) * HROWS, :], hcat_e[:])
                else:
                    nc.gpsimd.collective_compute(
                        "AllGather", OP.bypass, ins=[hcat_e[:]], outs=[hfulls[e][:]],
                        replica_groups=RG)

            # ======== phase B: dense edge-softmax aggregation (per encode)
            h1T_sb = [pp.tile([128, 2, NLOC], f32, tag=f"h1T{e}", name=f"h1T{e}") for e in range(2)]
            NPAR = [(MS + 1) // 2, MS // 2]  # m-count per parity
            for e in range(2):
                hfull_e = hfulls[e]
                # a_src columns [p, st] from the packed rows of each rank block
                asrc_bf = pm.tile([128, ST], bf16, tag="asrc_bf")
                for r in range(NCORE):
                    base = r * HROWS
                    for q in range(min(2, MS)):
                        npar = NPAR[q]
                        if npar == 0:
                            continue
                        nc.gpsimd.dma_start(
                            asrc_bf[:, r * MS + q:r * MS + 2 * npar - (1 - q):2],
                            hfull_e[base + NLOC:base + NLOC + npar,
                                    q * 128:(q + 1) * 128].rearrange("u p -> p u"))
                asrc_sb = pc.tile([128, ST], f32, tag=f"asrc{e}", name=f"asrc{e}")
                nc.vector.tensor_copy(asrc_sb[:], asrc_bf[:])
                # a_dst broadcast rows (own nodes)
                row = pm.tile([1, NLOC], f32, tag="adrow")
                nc.sync.dma_start(row[:], adsts[e][None, :])
                adst_bc = pc.tile([128, NLOC], f32, tag=f"adbc{e}", name=f"adbc{e}")
                for c in range(0, NLOC, 512):
                    cw = min(512, NLOC - c)
                    pbb = psum()
                    nc.tensor.matmul(pbb[:, :cw], ones1p[:], row[:, c:c + cw], start=True, stop=True)
                    nc.vector.tensor_copy(adst_bc[:, c:c + cw], pbb[:, :cw])

                pg = [psum() for _ in range(MS)]
                for st in range(ST):
                    r, sub = st // MS, st % MS
                    base = r * HROWS
                    ct = pb.tile([128, NLOC], bf16, tag="ct")
                    nc.sync.dma_start(ct[:], ct_s[st * 128:(st + 1) * 128, :])
                    rhs = pm.tile([128, HID + 1], bf16, tag="rhs")
                    nc.gpsimd.dma_start(rhs[:], hfull_e[base + sub * 128:base + sub * 128 + 128, 0:HID + 1])
                    Xt = pPp.tile([128, NLOC], bf16, tag="Xt")
                    if ABLATE == "nop":
                        pass
                    elif st % ACT_EVERY == 0:
                        Et = pPp.tile([128, NLOC], f32, tag="Et")
                        nc.scalar.activation(Et[:], adst_bc[:], AF.Prelu,
                                             bias=asrc_sb[:, st:st + 1], alpha=0.2)
                        nc.scalar.activation(Xt[:], Et[:], AF.Exp)
                    else:
                        Wt = pPp.tile([128, NLOC], f32, tag="Et")
                        nc.vector._custom_dve(LRELU_BIAS, out=Wt[:], in0=adst_bc[:],
                                              s0=asrc_sb[:, st:st + 1], imm2=0.2)
                        nc.scalar.activation(Xt[:], Wt[:], AF.Exp)
                    if ABLATE == "nop":
                        Pt = ct
                    else:
                        Pt = pPp.tile([128, NLOC], bf16, tag="Pt")
                        nc.vector.tensor_mul(Pt[:], Xt[:], ct[:])
                    for m in range(MS):
                        nc.tensor.matmul(
                            pg[m][:, :HID + 1],
                            Pt[:, m * 128:(m + 1) * 128],
                            rhs[:],
                            start=(st == 0), stop=(st == ST - 1),
                        )
                # evict: alpha-normalize, elu, transpose -> h1T
                for m in range(MS):
                    zeps = pe.tile([128, 1], f32, tag="zeps")
                    nc.vector.tensor_scalar_add(zeps[:], pg[m][:, HID:HID + 1], 1e-16)
                    rz = pe.tile([128, 1], f32, tag="rz")
                    nc.vector.reciprocal(rz[:], zeps[:])
                    nm = pe.tile([128, HID], f32, tag="nm")
                    nc.vector.tensor_scalar(nm[:], pg[m][:, 0:HID], rz[:], None, OP.mult)
                    emn = pe.tile([128, HID], f32, tag="emn")
                    nc.vector.tensor_scalar_min(emn[:], nm[:], 0.0)
                    nc.scalar.activation(emn[:], emn[:], AF.Exp)
                    epo = pe.tile([128, HID], f32, tag="epo")
                    nc.vector.tensor_scalar(epo[:], nm[:], 0.0, -1.0, OP.max, OP.add)
                    h1 = pe.tile([128, HID], f32, tag="h1")
                    nc.vector.tensor_add(h1[:], emn[:], epo[:])
                    for kt2 in range(2):
                        pt = psum()
                        nc.tensor.transpose(pt[:, :128], h1[:, kt2 * 128:(kt2 + 1) * 128], ident[:])
                        nc.vector.tensor_copy(h1T_sb[e][:, kt2, m * 128:(m + 1) * 128], pt[:, :128])

            # ======== phase C: h2 = h1 @ W2 (+ local transposes, AG#2 staging)
            h2T_sb = [pp.tile([OUT, NLOC], f32, tag=f"h2T{e}", name=f"h2T{e}") for e in range(2)]
            for e in range(2):
                for m in range(MS):
                    p2 = psum()
                    for kt2 in range(2):
                        nc.tensor.matmul(
                            p2[:, :OUT],
                            h1T_sb[e][:, kt2, m * 128:(m + 1) * 128],
                            W2_sb[:, kt2, :],
                            start=(kt2 == 0), stop=(kt2 == 1),
                        )
                    h2s = pe.tile([128, OUT], f32, tag="h2s")
                    nc.scalar.activation(h2s[:], p2[:, :OUT], AF.Copy)
                    nc.sync.dma_start(
                        (out_h2 if e == 0 else out_h2a)[m * 128:(m + 1) * 128, :], h2s[:])
                    h2b = pe.tile([128, OUT], bf16, tag="h2b")
                    nc.vector.tensor_copy(h2b[:], p2[:, :OUT])
                    nc.sync.dma_start(h2cat[m * 128:(m + 1) * 128, e * OUT:(e + 1) * OUT], h2b[:])
                    if e == 0:
                        nc.sync.dma_start(h2cat[m * 128:(m + 1) * 128, 128:129], onescol_bf[:])
                    ptr = psum()
                    nc.tensor.transpose(ptr[:OUT, :128], h2s[:, :], ident[:])
                    nc.vector.tensor_copy(h2T_sb[e][:, m * 128:(m + 1) * 128], ptr[:OUT, :128])

            # ======== AllGather #2
            if ABLATE == "nocc":
                for _r in range(NCORE):
                    nc.gpsimd.dma_start(h2full[_r * NLOC:(_r + 1) * NLOC, :], h2cat[:])
            else:
                nc.gpsimd.collective_compute(
                    "AllGather", OP.bypass, ins=[h2cat[:]], outs=[h2full[:]], replica_groups=RG)

            # ======== phase E: readout + discriminator
            prb = [psum() for _ in range((MS + 1) // 2)]
            pr = [prb[m // 2][:, (m % 2) * 256:(m % 2) * 256 + WH2] for m in range(MS)]
            for st in range(ST):
                gt = pb.tile([128, NLOC], bf16, tag="gt")
                nc.sync.dma_start(gt[:], gnT_s[st * 128:(st + 1) * 128, :])
                rr = pm.tile([128, WH2], bf16, tag="rr")
                nc.gpsimd.dma_start(rr[:], h2full[st * 128:(st + 1) * 128, :])
                for m in range(MS):
                    nc.tensor.matmul(
                        pr[m],
                        gt[:, m * 128:(m + 1) * 128],
                        rr[:],
                        start=(st == 0), stop=(st == ST - 1))
            # sweep 1: normalize g (Sqrt ops batched on ACT)
            gall = pp.tile([128, 2 * MS, OUT], f32)
            for m in range(MS):
                rm = pe.tile([128, 1], f32, tag="rm")
                nc.vector.reciprocal(rm[:], pr[m][:, 128:129])
                for gi in range(2):
                    g = gall[:, gi * MS + m, :]
                    nc.vector.tensor_scalar(g, pr[m][:, gi * OUT:(gi + 1) * OUT], rm[:], None, OP.mult)
                    sq = pe.tile([128, OUT], f32, tag="sq")
                    nc.vector.tensor_mul(sq[:], g, g)
                    ssn = pe.tile([128, 1], f32, tag="ssn")
                    nc.vector.reduce_sum(ssn[:], sq[:], axis=AX)
                    nc.scalar.activation(ssn[:], ssn[:], AF.Sqrt)
                    nc.vector.tensor_scalar_max(ssn[:], ssn[:], 1e-12)
                    nc.vector.reciprocal(ssn[:], ssn[:])
                    nc.vector.tensor_scalar(g, g, ssn[:], None, OP.mult)
            # sweep 2: one batched sigmoid
            nc.scalar.activation(gall[:], gall[:], AF.Sigmoid)
            # sweep 3: discriminator
            for m in range(MS):
                ts_ = []
                for e in range(2):
                    pd = psum()
                    nc.tensor.matmul(
                        pd[:, :OUT],
                        h2T_sb[e][:, m * 128:(m + 1) * 128],
                        discW_sb[:],
                        start=True, stop=True)
                    tt = pe.tile([128, OUT], f32, tag=f"tt{e}", name=f"tt{e}")
                    nc.vector.tensor_copy(tt[:], pd[:, :OUT])
                    ts_.append(tt)
                g0 = gall[:, 0 * MS + m, :]
                ga = gall[:, 1 * MS + m, :]
                prod = pe.tile([128, OUT], f32, tag="prod")
                retv = pe.tile([128, 2], f32, tag="retv")
                retav = pe.tile([128, 2], f32, tag="retav")
                for tv, gv, dtile, dcol in (
                        (ts_[0], g0, retv, 0), (ts_[1], g0, retv, 1),
                        (ts_[1], ga, retav, 0), (ts_[0], ga, retav, 1)):
                    nc.vector.tensor_mul(prod[:], tv[:], gv)
                    nc.vector.reduce_sum(dtile[:, dcol:dcol + 1], prod[:], axis=AX)
                nc.vector.tensor_scalar_add(retv[:], retv[:], db_bc[:])
                nc.vector.tensor_scalar_add(retav[:], retav[:], db_bc[:])
                nc.sync.dma_start(out_ret[m * 128:(m + 1) * 128, :], retv[:])
                nc.sync.dma_start(out_reta[m * 128:(m + 1) * 128, :], retav[:])

            # ======== phase D: decoder
            # rebuild h2^T (all nodes, bf16) by transposing gathered h2 blocks
            h2T_all = pp.tile([OUT, N], bf16)
            for stt in range(ST):
                h2f = pe.tile([128, OUT], bf16, tag="h2f")
                nc.gpsimd.dma_start(h2f[:], h2full[stt * 128:(stt + 1) * 128, 0:OUT])
                ptx = ps.tile([128, 512], bf16, tag="ps", name="pstb")
                nc.tensor.transpose(ptx[:OUT, :128], h2f[:], ident_bf[:])
                nc.vector.tensor_copy(h2T_all[:, stt * 128:(stt + 1) * 128], ptx[:OUT, :128])
            # streaming BN stats over all nodes
            NCHZ = [(c, min(512, N - c)) for c in range(0, N, 512)]
            nzch = len(NCHZ)
            acc_s = pc.tile([128, 2, nzch], f32)
            acc_q = pc.tile([128, 2, nzch], f32)
            for ci, (c0, cw) in enumerate(NCHZ):
                for m2 in range(2):
                    pz = psum()
                    nc.tensor.matmul(
                        pz[:, :cw],
                        Wd1b_sb[:, m2 * 128:(m2 + 1) * 128],
                        h2T_all[:, c0:c0 + cw],
                        start=True, stop=True)
                    nc.vector.reduce_sum(acc_s[:, m2, ci:ci + 1], pz[:, :cw], axis=AX)
                    sqs = pe.tile([128, 512], f32, tag="h3s")
                    nc.scalar.activation(sqs[:, :cw], pz[:, :cw], AF.Square,
                                         accum_out=acc_q[:, m2, ci:ci + 1])
            # stats -> scale/shift
            ssum = pc.tile([128, 2], f32)
            qsum = pc.tile([128, 2], f32)
            for m2 in range(2):
                nc.vector.reduce_sum(ssum[:, m2:m2 + 1], acc_s[:, m2, :], axis=AX)
                nc.vector.reduce_sum(qsum[:, m2:m2 + 1], acc_q[:, m2, :], axis=AX)
            mur = pc.tile([128, 2], f32)
            nc.vector.tensor_scalar_mul(mur[:], ssum[:], 1.0 / N)
            var = pc.tile([128, 2], f32)
            msq = pe.tile([128, 2], f32, tag="msq")
            nc.vector.tensor_mul(msq[:], mur[:], mur[:])
            nc.vector.tensor_scalar_mul(var[:], qsum[:], 1.0 / N)
            nc.vector.tensor_sub(var[:], var[:], msq[:])
            eps_col = pc.tile([128, 1], f32)
            nc.vector.memset(eps_col[:], EPS)
            std = pc.tile([128, 2], f32)
            nc.scalar.activation(std[:], var[:], AF.Sqrt, bias=eps_col[:])
            rs = pc.tile([128, 2], f32)
            nc.vector.reciprocal(rs[:], std[:])
            scl = pc.tile([128, 2], f32)
            nc.vector.tensor_mul(scl[:], rs[:], gamma_sb[:])
            sh = pc.tile([128, 2], f32)
            nc.vector.tensor_sub(sh[:], bd1_sb[:], mur[:])
            nc.vector.tensor_mul(sh[:], sh[:], scl[:])
            nc.vector.tensor_add(sh[:], sh[:], beta_sb[:])
            # own-block z, normalize, elu
            NCHL = [(c, min(512, NLOC - c)) for c in range(0, NLOC, 512)]
            znT = pp.tile([128, 2, NLOC], f32)
            for m2 in range(2):
                for c0, cw in NCHL:
                    pz = psum()
                    nc.tensor.matmul(
                        pz[:, :cw],
                        Wd1_sb[:, m2 * 128:(m2 + 1) * 128],
                        h2T_sb[0][:, c0:c0 + cw],
                        start=True, stop=True)
                    nc.vector.tensor_scalar(
                        znT[:, m2, c0:c0 + cw], pz[:, :cw],
                        scl[:, m2:m2 + 1], sh[:, m2:m2 + 1], OP.mult, OP.add)
            zel = pp.tile([128, 2, NLOC], bf16)
            tmp1 = pp.tile([128, 2, NLOC], f32)
            nc.vector.tensor_scalar_min(tmp1[:], znT[:], 0.0)
            nc.scalar.activation(tmp1[:], tmp1[:], AF.Exp)
            nc.vector.tensor_scalar(zel[:], znT[:], 0.0, -1.0, OP.max, OP.add)
            nc.vector.tensor_add(zel[:], zel[:], tmp1[:])
            # h3 = zel^T @ Wd2 + bd2
            Wd2r = Wd2bf.rearrange("(m p) i -> p m i", p=128)
            for c0, cw in NCH:
                wd2t = pm.tile([128, 2, 500], bf16, tag="wd2t")
                nc.sync.dma_start(wd2t[:, :, :cw], Wd2r[:, :, c0:c0 + cw])
                p3 = [psum() for _ in range(MS)]
                for m in range(MS):
                    for kt2 in range(2):
                        nc.tensor.matmul(
                            p3[m][:, :cw],
                            zel[:, kt2, m * 128:(m + 1) * 128],
                            wd2t[:, kt2, :cw],
                            start=(kt2 == 0), stop=(kt2 == 1))
                    h3s = pe.tile([128, 512], f32, tag="h3s")
                    nc.vector.tensor_add(h3s[:, :cw], p3[m][:, :cw], bd2_bc[:, c0:c0 + cw])
                    nc.sync.dma_start(out_h3[m * 128:(m + 1) * 128, c0:c0 + cw], h3s[:, :cw])

    nc.compile()
    return nc


# Need mybir at module level for the runner
import concourse.mybir as mybir  # noqa: E402


# ================================================================ host runner
class _SpmdRunner:
    def __init__(self, nc, n_cores):
        import jax
        from jax.sharding import Mesh, PartitionSpec, NamedSharding
        from jax.experimental.shard_map import shard_map
        from concourse.bass2jax import (
            _bass_exec_p, install_neuronx_cc_hook, partition_id_tensor)

        install_neuronx_cc_hook()
        self.jax = jax
        self.nc = nc
        self.n_cores = n_cores
        partition_name = nc.partition_id_tensor.name if nc.partition_id_tensor else None
        in_names, out_names, out_avals, zero_shapes = [], [], [], []
        for alloc in nc.m.functions[0].allocations:
            if not isinstance(alloc, mybir.MemoryLocationSet):
                continue
            name = alloc.memorylocations[0].name
            if alloc.kind == "ExternalInput":
                if name != partition_name:
                    in_names.append(name)
            elif alloc.kind == "ExternalOutput":
                shape = tuple(alloc.tensor_shape)
                dtype = mybir.dt.np(alloc.dtype)
                out_names.append(name)
                out_avals.append(jax.core.ShapedArray(shape, dtype))
                zero_shapes.append((shape, dtype))
        self.in_names, self.out_names = in_names, out_names
        self.out_avals, self.zero_shapes = out_avals, zero_shapes
        n_params, n_outs = len(in_names), len(out_avals)
        all_in = in_names + out_names + ([partition_name] if partition_name else [])

        def _body(*args):
            operands = list(args)
            if partition_name is not None:
                operands.append(partition_id_tensor())
            outs = _bass_exec_p.bind(
                *operands, out_avals=tuple(out_avals), in_names=tuple(all_in),
                out_names=tuple(out_names), lowering_input_output_aliases=(),
                sim_require_finite=True, sim_require_nnan=True, nc=nc)
            return tuple(outs)

        devices = jax.devices()[:n_cores]
        self.mesh = Mesh(np.asarray(devices), ("core",))
        in_specs = (PartitionSpec("core"),) * (n_params + n_outs)
        out_specs = (PartitionSpec("core"),) * n_outs
        self.fn = jax.jit(
            shard_map(_body, mesh=self.mesh, in_specs=in_specs,
                      out_specs=out_specs, check_rep=False),
            keep_unused=True)
        self.sharding = NamedSharding(self.mesh, PartitionSpec("core"))

    def put_inputs(self, in_maps):
        jax = self.jax
        args = []
        for name in self.in_names:
            if name == "dbg_addr" and name not in in_maps[0]:
                per = [np.zeros((1, 2), np.uint32)] * self.n_cores
            else:
                per = [np.asarray(in_maps[c][name]) for c in range(self.n_cores)]
            args.append(jax.device_put(np.concatenate(per, axis=0), self.sharding))
        for shape, dtype in self.zero_shapes:
            z = np.zeros((self.n_cores * shape[0], *shape[1:]), dtype)
            args.append(jax.device_put(z, self.sharding))
        return args

    def run(self, args):
        outs = self.fn(*args)
        self.jax.block_until_ready(outs)
        return outs

    def results(self, outs):
        res = []
        for c in range(self.n_cores):
            d = {}
            for i, name in enumerate(self.out_names):
                shape = self.out_avals[i].shape
                d[name] = np.asarray(outs[i]).reshape(self.n_cores, *shape)[c]
            res.append(d)
        return res


def _get_runner():
    global _RUNNER
    if _RUNNER is None:
        nc = _build_bass()
        _RUNNER = _SpmdRunner(nc, NCORE)
    return _RUNNER


# ================================================================ host-side prep
def _prep_in_maps(feat, feat_a, graph_neigh, W1, att_src, att_dst, W2, Wd1, bd1,
                  gamma, beta, Wd2, bd2, disc_W, disc_b, edge_index):
    feat = np.asarray(feat, np.float32)
    feat_a = np.asarray(feat_a, np.float32)
    gn = np.asarray(graph_neigh, np.float32)
    ei = np.asarray(edge_index).astype(np.int64)
    src, dst = ei[0], ei[1]

    # dense transposed edge-count matrix C^T[s, d]
    CT = np.zeros((N, N), np.float32)
    np.add.at(CT, (src, dst), 1.0)
    assert CT.max() < 256, "bf16 exact-int range exceeded"

    featT = np.ascontiguousarray(feat.T).astype(BF16)
    feataT = np.ascontiguousarray(feat_a.T).astype(BF16)
    gnT_bf = np.ascontiguousarray(gn.T).astype(BF16)

    att = np.concatenate([np.asarray(att_src, np.float32),
                          np.asarray(att_dst, np.float32)])
    common = {
        "W1bf": np.asarray(W1, np.float32).astype(BF16), "att": att,
        "W2": np.asarray(W2, np.float32), "Wd1": np.asarray(Wd1, np.float32),
        "Wd1bf": np.asarray(Wd1, np.float32).astype(BF16),
        "bd1": np.asarray(bd1, np.float32), "gamma": np.asarray(gamma, np.float32),
        "beta": np.asarray(beta, np.float32),
        "Wd2bf": np.asarray(Wd2, np.float32).astype(BF16),
        "bd2": np.asarray(bd2, np.float32), "discW": np.asarray(disc_W, np.float32),
        "discb": np.asarray(disc_b, np.float32).reshape(1),
    }
    in_maps = []
    for r in range(NCORE):
        sl = slice(r * NLOC, (r + 1) * NLOC)
        m = dict(common)
        m["featT0"] = np.ascontiguousarray(featT[:, sl])
        m["featT1"] = np.ascontiguousarray(feataT[:, sl])
        m["ct_s"] = np.ascontiguousarray(CT[:, sl]).astype(BF16)
        m["gnT_s"] = np.ascontiguousarray(gnT_bf[:, sl])
        in_maps.append(m)
    return in_maps


def kernel(**inputs):
    runner = _get_runner()
    in_maps = _prep_in_maps(**inputs)
    args = runner.put_inputs(in_maps)
    outs = runner.run(args)
    res = runner.results(outs)
    H2 = np.concatenate([res[r]["out_h2"] for r in range(NCORE)], axis=0)
    H2A = np.concatenate([res[r]["out_h2a"] for r in range(NCORE)], axis=0)
    H3 = np.concatenate([res[r]["out_h3"] for r in range(NCORE)], axis=0)
    RET = np.concatenate([res[r]["out_ret"] for r in range(NCORE)], axis=0)
    RETA = np.concatenate([res[r]["out_reta"] for r in range(NCORE)], axis=0)
    return (H2, H3, RET, RETA, H2, H2A)
